# revision 34
# baseline (speedup 1.0000x reference)
"""AdaptiveSpectrumLayer Trainium2 kernel — 8-core pure data parallel.

Pipeline per core (B_local=8 batches, COLS=1024 columns = (b,f)):
  rfft (DFT matmuls, f32r) -> mag/s/c features (no trig: s=im/mag, c=re/mag)
  -> per-freq 4->32->2 relu MLP (block-diag bf16 matmuls, 32 freqs/group)
  -> collapsed gate matmuls (bf16) -> sigmoid weights
  -> blend -> irfft (DFT matmuls, f32r)
"""
import sys
import numpy as np

sys.path.insert(0, "/opt/trn_rl_repo")

import ml_dtypes
from contextlib import ExitStack

import concourse.bass as bass
import concourse.tile as tile
from concourse import mybir
from concourse import bacc
from concourse.bass_utils import run_bass_kernel_spmd


def _ensure_ntff_hook():
    """The agent image's antenv lacks axon_hooks; inject a stub and register
    the ctypes NTFF profiling hook so trace=True works. Safe no-op if parts
    are missing."""
    try:
        import antenv.axon_hooks  # noqa: F401
        return
    except ImportError:
        pass
    try:
        import types
        import antenv
        mod = types.ModuleType("antenv.axon_hooks")
        _state = {"hook": None}
        mod.set_axon_ntff_profile_hook = lambda h: _state.__setitem__("hook", h)
        mod.get_axon_ntff_profile_hook = lambda: _state["hook"]
        sys.modules["antenv.axon_hooks"] = mod
        antenv.axon_hooks = mod
        from trn_agent_boot.trn_boot import _ntff_profile_via_ctypes
        so = "/opt/axon/libaxon_pjrt.so"
        import os
        if os.path.exists(so):
            mod.set_axon_ntff_profile_hook(_ntff_profile_via_ctypes(so))
    except Exception:
        pass


_ensure_ntff_hook()

# ---- problem constants (hardcoded; kernel.py must be self-contained) ----
B, H, F, HID = 64, 512, 128, 32
FS = 100.0
NF = H // 2 + 1          # 257
NFP = 288                # padded freq count: 9 groups of 32 = 3 chunks of 96
NG = 9                   # freq groups (32 each)
NCH = 3                  # freq chunks (96 each)
CPW = 96                 # chunk width
NCORE = 8
BL = B // NCORE          # 8
COLS = BL * F            # 1024
NC2 = 2                  # 512-wide N chunks per 1024 cols
EPS = 1e-30

f32 = mybir.dt.float32
f32r = mybir.dt.float32r
bf16 = mybir.dt.bfloat16
AF = mybir.ActivationFunctionType
ALU = mybir.AluOpType


# =========================================================================
# Host-side weight preparation
# =========================================================================
def build_host_weights(Wp, bp, Wg, bg, Wm, bm, Wph, bph):
    freqs = np.fft.rfftfreq(H, 1.0 / FS)[:NF].astype(np.float32)

    n_idx = np.arange(NFP)
    t_idx = np.arange(H)
    valid = (n_idx < NF).astype(np.float32)
    theta = 2.0 * np.pi * np.outer(t_idx, n_idx) / H  # (512, 288)
    inv_sqrt_h = 1.0 / np.sqrt(H)

    RC = (np.cos(theta) * inv_sqrt_h * valid[None, :]).astype(np.float32)
    RS = (-np.sin(theta) * inv_sqrt_h * valid[None, :]).astype(np.float32)
    w_n = np.where((n_idx == 0) | (n_idx == 256), 1.0, 2.0) * valid
    IC = (np.cos(theta) * inv_sqrt_h * w_n[None, :]).astype(np.float32)
    IS = (-np.sin(theta) * inv_sqrt_h * w_n[None, :]).astype(np.float32)

    WpP = np.zeros((NFP, 4, HID), np.float32); WpP[:NF] = Wp
    bpP = np.zeros((NFP, HID), np.float32);    bpP[:NF] = bp
    WmP = np.zeros((NFP, HID), np.float32);    WmP[:NF] = Wm
    bmP = np.zeros((NFP,), np.float32);        bmP[:NF] = bm
    WphP = np.zeros((NFP, HID), np.float32);   WphP[:NF] = Wph
    bphP = np.zeros((NFP,), np.float32);       bphP[:NF] = bph
    fP = np.zeros((NFP,), np.float32);         fP[:NF] = freqs

    D = fP[:, None] * WpP[:, 3, :] + bpP  # (288, 32)

    WgR = Wg.reshape(NF, HID, NF)
    G = np.zeros((3, NFP, NFP), np.float32)
    for f in range(3):
        G[f, :NF, :NF] = np.einsum("nh,nhj->nj", Wp[:, f, :], WgR)
    gconst = np.zeros((NFP,), np.float32)
    gconst[:NF] = np.einsum("nh,nhj->j", D[:NF], WgR) + bg

    # ---- device layouts ----
    # w_rfft (128, 4, 2, 3, 96): [tp][kt][ri][ch][fc]
    w_rfft = np.zeros((128, 4, 2, NCH, CPW), np.float32)
    RCr = RC.reshape(4, 128, NCH, CPW)  # [kt][tp][ch][fc]
    RSr = RS.reshape(4, 128, NCH, CPW)
    w_rfft[:, :, 0] = RCr.transpose(1, 0, 2, 3)
    w_rfft[:, :, 1] = RSr.transpose(1, 0, 2, 3)

    # w_proj (128, 9, 8, 128) bf16
    w_proj = np.zeros((128, NG, 8, 128), np.float32)
    ii = np.arange(32)
    for g in range(NG):
        n = 32 * g + ii  # (32,)
        for f in range(3):
            feat = WpP[n, f, :]  # (32, 32) [i, h]
            # rows 32f+i, cols 4i+hh = feat[i, 4j+hh]
            for j in range(8):
                blk = feat[:, 4 * j:4 * j + 4]  # (32 i, 4 hh)
                for hh in range(4):
                    w_proj[32 * f + ii, g, j, 4 * ii + hh] = blk[:, hh]

    # d_bias (128, 9, 8): [4i+hh][g][j]
    d_bias = np.zeros((128, NG, 8), np.float32)
    ii = np.arange(32)
    for g in range(NG):
        n = 32 * g + ii
        for j in range(8):
            for hh in range(4):
                d_bias[4 * ii + hh, g, j] = D[n, 4 * j + hh]

    # w_red (128, 9, 8, 64): [4i+hh][g][j][col]
    w_red = np.zeros((128, NG, 8, 64), np.float32)
    for g in range(NG):
        n = 32 * g + ii
        for j in range(8):
            for hh in range(4):
                w_red[4 * ii + hh, g, j, ii] = WmP[n, 4 * j + hh]
                w_red[4 * ii + hh, g, j, 32 + ii] = WphP[n, 4 * j + hh]

    # mp_bias (64, 9)
    mp_bias = np.zeros((64, NG), np.float32)
    for g in range(NG):
        n = 32 * g + ii
        mp_bias[ii, g] = bmP[n]
        mp_bias[32 + ii, g] = bphP[n]

    # w_gate (128, 9, 3, 96): [32f+i][g][jt][jc] = G[f][32g+i, 96jt+jc]
    w_gate = np.zeros((128, NG, NCH, CPW), np.float32)
    for g in range(NG):
        n = 32 * g + ii
        for f in range(3):
            Gr = G[f][n].reshape(32, NCH, CPW)  # [i][jt][jc]
            w_gate[32 * f + ii, g] = Gr

    # bg_bias (96, 3): [jc][jt]
    bg_bias = gconst.reshape(NCH, CPW).T.copy()

    # w_irfft (96, 2, 3, 4, 128): [p][ri][ch][mt][tc]
    w_irfft = np.zeros((CPW, 2, NCH, 4, 128), np.float32)
    ICr = IC.reshape(4, 128, NCH, CPW)  # [mt][tc][ch][p]
    ISr = IS.reshape(4, 128, NCH, CPW)
    w_irfft[:, 0] = ICr.transpose(3, 2, 0, 1)
    w_irfft[:, 1] = ISr.transpose(3, 2, 0, 1)

    tobf = lambda a: a.astype(ml_dtypes.bfloat16)
    return dict(
        w_rfft=tobf(w_rfft),
        w_proj=tobf(w_proj),
        d_bias=d_bias,
        w_red=tobf(w_red),
        mp_bias=mp_bias,
        w_gate=tobf(w_gate),
        bg_bias=bg_bias,
        w_irfft=tobf(w_irfft),
    )


# =========================================================================
# Device kernel builder
# =========================================================================
def pstride_ap(t, g3_free_idx, part_start, part_step, part_num):
    """AP over tile t: partitions part_start + part_step*i (i<part_num),
    free dims sliced by g3_free_idx (an AP from regular indexing)."""
    base = g3_free_idx
    return bass.AP(
        tensor=base.tensor,
        offset=base.offset,
        ap=[[part_step, part_num]] + list(base.ap[1:]),
    )


def build_kernel():
    nc = bacc.Bacc()

    x_d = nc.declare_dram_parameter("x", [BL, H, F], f32, isOutput=False)
    w_rfft_d = nc.declare_dram_parameter("w_rfft", [128, 4, 2, NCH, CPW], bf16, isOutput=False)
    w_proj_d = nc.declare_dram_parameter("w_proj", [128, NG, 8, 128], bf16, isOutput=False)
    d_bias_d = nc.declare_dram_parameter("d_bias", [128, NG, 8], f32, isOutput=False)
    w_red_d = nc.declare_dram_parameter("w_red", [128, NG, 8, 64], bf16, isOutput=False)
    mp_bias_d = nc.declare_dram_parameter("mp_bias", [64, NG], f32, isOutput=False)
    w_gate_d = nc.declare_dram_parameter("w_gate", [128, NG, NCH, CPW], bf16, isOutput=False)
    bg_bias_d = nc.declare_dram_parameter("bg_bias", [CPW, NCH], f32, isOutput=False)
    w_irfft_d = nc.declare_dram_parameter("w_irfft", [CPW, 2, NCH, 4, 128], bf16, isOutput=False)
    out_d = nc.declare_dram_parameter("out", [BL, H, F], f32, isOutput=True)

    with tile.TileContext(nc) as tc, ExitStack() as ctx:
        consts = ctx.enter_context(tc.tile_pool(name="consts", bufs=1))
        bigio = ctx.enter_context(tc.tile_pool(name="bigio", bufs=1))
        scratch = ctx.enter_context(tc.tile_pool(name="scratch", bufs=2))
        xr_pool = ctx.enter_context(tc.tile_pool(name="xr", bufs=4))
        mp_pool = ctx.enter_context(tc.tile_pool(name="mp", bufs=2))
        gf_pool = ctx.enter_context(tc.tile_pool(name="gf", bufs=2))
        bl_pool = ctx.enter_context(tc.tile_pool(name="bl", bufs=2))

        ps_a = ctx.enter_context(tc.tile_pool(name="ps_a", bufs=3, space="PSUM"))
        ps_proj = ctx.enter_context(tc.tile_pool(name="ps_proj", bufs=3, space="PSUM"))
        ps_red = ctx.enter_context(tc.tile_pool(name="ps_red", bufs=2, space="PSUM"))

        # ---- persistent SBUF tensors ----
        x_sb = bigio.tile([128, 4, BL, F], f32, tag="xout")
        w_rfft_sb = consts.tile([128, 4, 2, NCH, CPW], bf16, tag="w_rfft")
        w_proj_sb = consts.tile([128, NG, 8, 128], bf16, tag="w_proj")
        d_bias_sb = consts.tile([128, NG, 8], f32, tag="d_bias")
        w_red_sb = consts.tile([128, NG, 8, 64], bf16, tag="w_red")
        mp_bias_sb = consts.tile([64, NG], f32, tag="mp_bias")
        w_gate_sb = consts.tile([128, NG, NCH, CPW], bf16, tag="w_gate")
        bg_bias_sb = consts.tile([CPW, NCH], f32, tag="bg_bias")
        w_irfft_sb = consts.tile([CPW, 2, NCH, 4, 128], bf16, tag="w_irfft")

        re_sb = consts.tile([CPW, NCH, COLS], f32, tag="re")
        im_sb = consts.tile([CPW, NCH, COLS], f32, tag="im")
        x_bf = consts.tile([128, 4, BL, F], bf16, tag="x_bf")
        bre_sb = consts.tile([CPW, NCH, COLS], bf16, tag="bre")
        bim_sb = consts.tile([CPW, NCH, COLS], bf16, tag="bim")
        ff_sb = consts.tile([128, NG, COLS], bf16, tag="ff")
        m_sb = consts.tile([CPW, NCH, COLS], f32, tag="m_t")
        ph_sb = consts.tile([CPW, NCH, COLS], f32, tag="ph_t")
        w_sb = consts.tile([CPW, NCH, COLS], f32, tag="w_t")

        # ---- load weights + input ----
        x_r = x_d[:].rearrange("b (kt p) f -> kt p b f", p=128)
        nc.sync.dma_start(out=w_rfft_sb, in_=w_rfft_d[:])
        for kt in range(4):
            nc.sync.dma_start(out=x_sb[:, kt], in_=x_r[kt])
            nc.vector.tensor_copy(out=x_bf[:, kt], in_=x_sb[:, kt])
        nc.sync.dma_start(out=w_proj_sb, in_=w_proj_d[:])
        nc.sync.dma_start(out=d_bias_sb, in_=d_bias_d[:])
        nc.sync.dma_start(out=w_red_sb, in_=w_red_d[:])
        nc.sync.dma_start(out=mp_bias_sb, in_=mp_bias_d[:])
        nc.sync.dma_start(out=w_gate_sb, in_=w_gate_d[:])
        nc.sync.dma_start(out=bg_bias_sb, in_=bg_bias_d[:])
        nc.sync.dma_start(out=w_irfft_sb, in_=w_irfft_d[:])

        # zero FF once (4th rows are never written; junk*0 = NaN risk otherwise)
        nc.vector.memset(ff_sb[:], 0.0)

        # constant bias columns for activations
        def const_col(value, tag):
            t = consts.tile([128, 1], f32, tag=tag)
            nc.vector.memset(t, value)
            return t

        eps_c = const_col(EPS, "c_eps")
        hpi_c = const_col(float(np.pi / 2.0), "c_hpi")
        pi_c = const_col(float(np.pi), "c_pi")

        # ================= rfft =================
        for ch in range(NCH):
            for ri in range(2):
                dst = re_sb if ri == 0 else im_sb
                for nck in range(NC2):
                    pt_f = ps_a.tile([128, 512], f32, tag="ps_a", name="ps_a")
                    pt = pt_f[:CPW]
                    for kt in range(4):
                        nc.tensor.matmul(
                            out=pt,
                            lhsT=w_rfft_sb[:, kt, ri, ch, :],
                            rhs=x_bf[:, kt, 4 * nck:4 * (nck + 1), :],
                            start=(kt == 0),
                            stop=(kt == 3),
                        )
                    nc.vector.tensor_copy(
                        out=dst[:, ch, 512 * nck:512 * (nck + 1)], in_=pt
                    )

        # ================= features: mag, s, c -> FF (interleaved) =============
        for ch in range(NCH):
            for nck in range(NC2):
                cs = slice(512 * nck, 512 * (nck + 1))
                re_c = re_sb[:, ch, cs]
                im_c = im_sb[:, ch, cs]
                msq = scratch.tile([CPW, 512], f32, tag="msq")
                t2 = scratch.tile([CPW, 512], f32, tag="tmpb")
                nc.vector.tensor_mul(out=msq, in0=re_c, in1=re_c)
                nc.vector.tensor_mul(out=t2, in0=im_c, in1=im_c)
                nc.vector.tensor_add(out=msq, in0=msq, in1=t2)
                magf = scratch.tile([CPW, 512], f32, tag="magf")
                nc.scalar.activation(out=magf, in_=msq, func=AF.Sqrt,
                                     bias=eps_c[:CPW], scale=1.0)
                rr = scratch.tile([CPW, 512], f32, tag="tmpb", name="rr")
                nc.vector.reciprocal(out=rr, in_=magf)
                magb = scratch.tile([CPW, 512], bf16, tag="magb")
                sbf = scratch.tile([CPW, 512], bf16, tag="sbf")
                cbf = scratch.tile([CPW, 512], bf16, tag="cbf")
                nc.vector.tensor_copy(out=magb, in_=magf)
                nc.vector.tensor_mul(out=sbf, in0=im_c, in1=rr)
                nc.vector.tensor_mul(out=cbf, in0=re_c, in1=rr)
                # interleave into FF via SBUF->SBUF DMA (cross-partition)
                for q in range(3):  # groups 3*ch + q, rows 32q..32q+32
                    g = 3 * ch + q
                    for f, srct in enumerate((magb, sbf, cbf)):
                        nc.gpsimd.tensor_copy(
                            out=ff_sb[32 * f:32 * f + 32, g, cs],
                            in_=srct[32 * q:32 * q + 32, :])

        # ================= per-freq MLP: proj -> relu -> reduce =================
        for g in range(NG):
            ch, p0 = g // 3, 32 * (g % 3)
            for nck in range(NC2):
                red_pt = ps_red.tile([64, 512], f32, tag="ps_red")
                for j in range(8):
                    proj_pt = ps_proj.tile([128, 512], f32, tag="ps_proj")
                    nc.tensor.matmul(
                        out=proj_pt,
                        lhsT=w_proj_sb[:, g, j, :],
                        rhs=ff_sb[:, g, 512 * nck:512 * (nck + 1)],
                        start=True, stop=True,
                    )
                    xr = xr_pool.tile([128, 512], bf16, tag="xr")
                    if j % 2 == 0:
                        nc.scalar.activation(
                            out=xr, in_=proj_pt, func=AF.Relu,
                            bias=d_bias_sb[:, g, j:j + 1], scale=1.0,
                        )
                    else:
                        nc.vector.tensor_scalar(
                            out=xr, in0=proj_pt,
                            scalar1=d_bias_sb[:, g, j:j + 1], scalar2=0.0,
                            op0=ALU.add, op1=ALU.max,
                        )
                    nc.tensor.matmul(
                        out=red_pt,
                        lhsT=w_red_sb[:, g, j, :],
                        rhs=xr,
                        start=(j == 0), stop=(j == 7),
                    )
                # m / ph activations + relocation
                mp = mp_pool.tile([64, 512], f32, tag="mp")
                nc.scalar.activation(
                    out=mp[0:32, :], in_=red_pt[0:32, :], func=AF.Relu,
                    bias=mp_bias_sb[0:32, g:g + 1], scale=1.0,
                )
                nc.scalar.activation(
                    out=mp[32:64, :], in_=red_pt[32:64, :], func=AF.Sigmoid,
                    bias=mp_bias_sb[32:64, g:g + 1], scale=1.0,
                )
                nc.gpsimd.tensor_copy(
                    out=m_sb[p0:p0 + 32, ch, 512 * nck:512 * (nck + 1)],
                    in_=mp[0:32, :],
                )
                nc.gpsimd.tensor_copy(
                    out=ph_sb[p0:p0 + 32, ch, 512 * nck:512 * (nck + 1)],
                    in_=mp[32:64, :],
                )

        # ================= gate =================
        for jt in range(NCH):
            for nck in range(NC2):
                gp_f = ps_a.tile([128, 512], f32, tag="ps_a", name="ps_a")
                gp = gp_f[:CPW]
                for g in range(NG):
                    nc.tensor.matmul(
                        out=gp,
                        lhsT=w_gate_sb[:, g, jt, :],
                        rhs=ff_sb[:, g, 512 * nck:512 * (nck + 1)],
                        start=(g == 0), stop=(g == NG - 1),
                    )
                gt = gf_pool.tile([CPW, 512], f32, tag="gt")
                sg = gf_pool.tile([CPW, 512], f32, tag="sg")
                nc.scalar.activation(out=gt, in_=gp, func=AF.Identity,
                                     bias=bg_bias_sb[:, jt:jt + 1], scale=1.0)
                nc.scalar.activation(out=sg, in_=gp, func=AF.Sigmoid,
                                     bias=bg_bias_sb[:, jt:jt + 1], scale=1.0)
                nc.vector.tensor_mul(out=gt, in0=gt, in1=sg)
                nc.scalar.activation(
                    out=w_sb[:, jt, 512 * nck:512 * (nck + 1)],
                    in_=gt, func=AF.Sigmoid, bias=0.0, scale=1.0,
                )

        # ================= blend =================
        TWO_PI = float(2.0 * np.pi)
        PI = float(np.pi)
        HALF_PI = float(np.pi / 2.0)
        for ch in range(NCH):
            for nck in range(NC2):
                cs = slice(512 * nck, 512 * (nck + 1))
                ph_c = ph_sb[:, ch, cs]
                m_c = m_sb[:, ch, cs]
                w_c = w_sb[:, ch, cs]
                cosph = bl_pool.tile([CPW, 512], f32, tag="cosph")
                sinph = bl_pool.tile([CPW, 512], f32, tag="sinph")
                # Sin spline is only valid on [-pi, pi].
                # sin(2pi u) = sin(pi - 2pi u), arg in [-pi, pi] for u in [0,1].
                # cos(2pi u) = sin(2pi w), w = (u + 0.25) mod 1, same formula.
                # cos(2pi u) = 1 - 2 sin^2(pi u), arg pi*u in [0, pi].
                shalf = bl_pool.tile([CPW, 512], f32, tag="shalf")
                nc.scalar.activation(out=shalf, in_=ph_c, func=AF.Sin,
                                     bias=0.0, scale=PI)
                nc.vector.tensor_mul(out=shalf, in0=shalf, in1=shalf)
                nc.vector.tensor_scalar(out=cosph, in0=shalf, scalar1=-2.0,
                                        scalar2=1.0, op0=ALU.mult, op1=ALU.add)
                nc.scalar.activation(out=sinph, in_=ph_c, func=AF.Sin,
                                     bias=pi_c[:CPW], scale=-TWO_PI)
                for trig, src_t, dst_t in ((cosph, re_sb, bre_sb),
                                           (sinph, im_sb, bim_sb)):
                    d_c = src_t[:, ch, cs]
                    t1 = bl_pool.tile([CPW, 512], f32, tag="t1")
                    nc.vector.tensor_mul(out=t1, in0=m_c, in1=trig)   # m*cos
                    nc.vector.tensor_sub(out=t1, in0=t1, in1=d_c)     # - re
                    nc.vector.tensor_mul(out=t1, in0=t1, in1=w_c)     # * w
                    nc.vector.tensor_add(out=dst_t[:, ch, cs], in0=d_c, in1=t1)

        # ================= irfft =================
        out_sb = bigio.tile([128, 4, BL, F], f32, tag="xout")
        for mt in range(4):
            for nck in range(NC2):
                pt = ps_a.tile([128, 512], f32, tag="ps_a")
                k = 0
                for ri, src in enumerate((bre_sb, bim_sb)):
                    for ch in range(NCH):
                        nc.tensor.matmul(
                            out=pt,
                            lhsT=w_irfft_sb[:, ri, ch, mt, :],
                            rhs=src[:, ch, 512 * nck:512 * (nck + 1)],
                            start=(k == 0), stop=(k == 5),
                        )
                        k += 1
                nc.scalar.copy(out=out_sb[:, mt, 4 * nck:4 * (nck + 1), :], in_=pt)

        out_r = out_d[:].rearrange("b (mt p) f -> mt p b f", p=128)
        for mt in range(4):
            nc.sync.dma_start(out=out_r[mt], in_=out_sb[:, mt])

    nc.finalize()
    return nc


_CACHE = {}


def _get_nc():
    if "nc" not in _CACHE:
        _CACHE["nc"] = build_kernel()
    return _CACHE["nc"]


def kernel(x, Wp, bp, Wg, bg, Wm, bm, Wph, bph, _trace=False):
    x = np.ascontiguousarray(np.asarray(x, dtype=np.float32))
    hw = build_host_weights(
        np.asarray(Wp, np.float32), np.asarray(bp, np.float32),
        np.asarray(Wg, np.float32), np.asarray(bg, np.float32),
        np.asarray(Wm, np.float32), np.asarray(bm, np.float32),
        np.asarray(Wph, np.float32), np.asarray(bph, np.float32),
    )
    nc = _get_nc()
    in_maps = []
    for i in range(NCORE):
        m = {"x": np.ascontiguousarray(x[i * BL:(i + 1) * BL])}
        m.update(hw)
        in_maps.append(m)
    res = run_bass_kernel_spmd(nc, in_maps, core_ids=list(range(NCORE)),
                               trace=_trace)
    out = np.concatenate([np.asarray(r["out"]) for r in res.results], axis=0)
    if _trace:
        _CACHE["last_exec_time_ns"] = res.exec_time_ns
        _CACHE["last_results"] = res
    return out.astype(np.float32)


# revision 38
# speedup vs baseline: 1.4670x; 1.4670x over previous
"""AdaptiveSpectrumLayer Trainium2 kernel — 8-core pure data parallel.

Pipeline per core (B_local=8 batches, COLS=1024 columns = (b,f)):
  rfft (DFT matmuls, f32r) -> mag/s/c features (no trig: s=im/mag, c=re/mag)
  -> per-freq 4->32->2 relu MLP (block-diag bf16 matmuls, 32 freqs/group)
  -> collapsed gate matmuls (bf16) -> sigmoid weights
  -> blend -> irfft (DFT matmuls, f32r)
"""
import sys
import numpy as np

sys.path.insert(0, "/opt/trn_rl_repo")

import ml_dtypes
from contextlib import ExitStack

import concourse.bass as bass
import concourse.tile as tile
from concourse import mybir
from concourse import bacc
from concourse.bass_utils import run_bass_kernel_spmd


def _ensure_ntff_hook():
    """The agent image's antenv lacks axon_hooks; inject a stub and register
    the ctypes NTFF profiling hook so trace=True works. Safe no-op if parts
    are missing."""
    try:
        import antenv.axon_hooks  # noqa: F401
        return
    except ImportError:
        pass
    try:
        import types
        import antenv
        mod = types.ModuleType("antenv.axon_hooks")
        _state = {"hook": None}
        mod.set_axon_ntff_profile_hook = lambda h: _state.__setitem__("hook", h)
        mod.get_axon_ntff_profile_hook = lambda: _state["hook"]
        sys.modules["antenv.axon_hooks"] = mod
        antenv.axon_hooks = mod
        from trn_agent_boot.trn_boot import _ntff_profile_via_ctypes
        so = "/opt/axon/libaxon_pjrt.so"
        import os
        if os.path.exists(so):
            mod.set_axon_ntff_profile_hook(_ntff_profile_via_ctypes(so))
    except Exception:
        pass


_ensure_ntff_hook()

# ---- problem constants (hardcoded; kernel.py must be self-contained) ----
B, H, F, HID = 64, 512, 128, 32
FS = 100.0
NF = H // 2 + 1          # 257
NFP = 288                # padded freq count: 9 groups of 32 = 3 chunks of 96
NG = 9                   # freq groups (32 each)
NCH = 3                  # freq chunks (96 each)
CPW = 96                 # chunk width
NCORE = 8
BL = B // NCORE          # 8
COLS = BL * F            # 1024
NC2 = 2                  # 512-wide N chunks per 1024 cols
EPS = 1e-30

f32 = mybir.dt.float32
f32r = mybir.dt.float32r
bf16 = mybir.dt.bfloat16
AF = mybir.ActivationFunctionType
ALU = mybir.AluOpType


# =========================================================================
# Host-side weight preparation
# =========================================================================
def build_host_weights(Wp, bp, Wg, bg, Wm, bm, Wph, bph):
    freqs = np.fft.rfftfreq(H, 1.0 / FS)[:NF].astype(np.float32)

    n_idx = np.arange(NFP)
    t_idx = np.arange(H)
    valid = (n_idx < NF).astype(np.float32)
    theta = 2.0 * np.pi * np.outer(t_idx, n_idx) / H  # (512, 288)
    inv_sqrt_h = 1.0 / np.sqrt(H)

    RC = (np.cos(theta) * inv_sqrt_h * valid[None, :]).astype(np.float32)
    RS = (-np.sin(theta) * inv_sqrt_h * valid[None, :]).astype(np.float32)
    w_n = np.where((n_idx == 0) | (n_idx == 256), 1.0, 2.0) * valid
    IC = (np.cos(theta) * inv_sqrt_h * w_n[None, :]).astype(np.float32)
    IS = (-np.sin(theta) * inv_sqrt_h * w_n[None, :]).astype(np.float32)

    WpP = np.zeros((NFP, 4, HID), np.float32); WpP[:NF] = Wp
    bpP = np.zeros((NFP, HID), np.float32);    bpP[:NF] = bp
    WmP = np.zeros((NFP, HID), np.float32);    WmP[:NF] = Wm
    bmP = np.zeros((NFP,), np.float32);        bmP[:NF] = bm
    WphP = np.zeros((NFP, HID), np.float32);   WphP[:NF] = Wph
    bphP = np.zeros((NFP,), np.float32);       bphP[:NF] = bph
    fP = np.zeros((NFP,), np.float32);         fP[:NF] = freqs

    D = fP[:, None] * WpP[:, 3, :] + bpP  # (288, 32)

    WgR = Wg.reshape(NF, HID, NF)
    G = np.zeros((3, NFP, NFP), np.float32)
    for f in range(3):
        G[f, :NF, :NF] = np.einsum("nh,nhj->nj", Wp[:, f, :], WgR)
    gconst = np.zeros((NFP,), np.float32)
    gconst[:NF] = np.einsum("nh,nhj->j", D[:NF], WgR) + bg

    # ---- device layouts ----
    # w_rfft (128, 4, 2, 3, 96): [tp][kt][ri][ch][fc]
    w_rfft = np.zeros((128, 4, 2, NCH, CPW), np.float32)
    RCr = RC.reshape(4, 128, NCH, CPW)  # [kt][tp][ch][fc]
    RSr = RS.reshape(4, 128, NCH, CPW)
    w_rfft[:, :, 0] = RCr.transpose(1, 0, 2, 3)
    w_rfft[:, :, 1] = RSr.transpose(1, 0, 2, 3)

    # w_proj (128, 9, 8, 128) bf16
    w_proj = np.zeros((128, NG, 8, 128), np.float32)
    ii = np.arange(32)
    for g in range(NG):
        n = 32 * g + ii  # (32,)
        for f in range(3):
            feat = WpP[n, f, :]  # (32, 32) [i, h]
            # rows 32f+i, cols 4i+hh = feat[i, 4j+hh]
            for j in range(8):
                blk = feat[:, 4 * j:4 * j + 4]  # (32 i, 4 hh)
                for hh in range(4):
                    w_proj[32 * f + ii, g, j, 4 * ii + hh] = blk[:, hh]

    # d_bias (128, 9, 8): [4i+hh][g][j]
    d_bias = np.zeros((128, NG, 8), np.float32)
    ii = np.arange(32)
    for g in range(NG):
        n = 32 * g + ii
        for j in range(8):
            for hh in range(4):
                d_bias[4 * ii + hh, g, j] = D[n, 4 * j + hh]

    # w_red (128, 9, 8, 64): [4i+hh][g][j][col]
    w_red = np.zeros((128, NG, 8, 64), np.float32)
    for g in range(NG):
        n = 32 * g + ii
        for j in range(8):
            for hh in range(4):
                w_red[4 * ii + hh, g, j, ii] = WmP[n, 4 * j + hh]
                w_red[4 * ii + hh, g, j, 32 + ii] = WphP[n, 4 * j + hh]

    # mp_bias (64, 9)
    mp_bias = np.zeros((64, NG), np.float32)
    for g in range(NG):
        n = 32 * g + ii
        mp_bias[ii, g] = bmP[n]
        mp_bias[32 + ii, g] = bphP[n]

    # w_gate (128, 9, 3, 96): [32f+i][g][jt][jc] = G[f][32g+i, 96jt+jc]
    w_gate = np.zeros((128, NG, NCH, CPW), np.float32)
    for g in range(NG):
        n = 32 * g + ii
        for f in range(3):
            Gr = G[f][n].reshape(32, NCH, CPW)  # [i][jt][jc]
            w_gate[32 * f + ii, g] = Gr

    # bg_bias (96, 3): [jc][jt]
    bg_bias = gconst.reshape(NCH, CPW).T.copy()

    # w_irfft (96, 2, 3, 4, 128): [p][ri][ch][mt][tc]
    w_irfft = np.zeros((CPW, 2, NCH, 4, 128), np.float32)
    ICr = IC.reshape(4, 128, NCH, CPW)  # [mt][tc][ch][p]
    ISr = IS.reshape(4, 128, NCH, CPW)
    w_irfft[:, 0] = ICr.transpose(3, 2, 0, 1)
    w_irfft[:, 1] = ISr.transpose(3, 2, 0, 1)

    tobf = lambda a: a.astype(ml_dtypes.bfloat16)
    return dict(
        w_rfft=tobf(w_rfft),
        w_proj=tobf(w_proj),
        d_bias=d_bias,
        w_red=tobf(w_red),
        mp_bias=mp_bias,
        w_gate=tobf(w_gate),
        bg_bias=bg_bias,
        w_irfft=tobf(w_irfft),
    )


# =========================================================================
# Device kernel builder
# =========================================================================
def pstride_ap(t, g3_free_idx, part_start, part_step, part_num):
    """AP over tile t: partitions part_start + part_step*i (i<part_num),
    free dims sliced by g3_free_idx (an AP from regular indexing)."""
    base = g3_free_idx
    return bass.AP(
        tensor=base.tensor,
        offset=base.offset,
        ap=[[part_step, part_num]] + list(base.ap[1:]),
    )


def build_kernel():
    nc = bacc.Bacc()

    x_d = nc.declare_dram_parameter("x", [BL, H, F], f32, isOutput=False)
    w_rfft_d = nc.declare_dram_parameter("w_rfft", [128, 4, 2, NCH, CPW], bf16, isOutput=False)
    w_proj_d = nc.declare_dram_parameter("w_proj", [128, NG, 8, 128], bf16, isOutput=False)
    d_bias_d = nc.declare_dram_parameter("d_bias", [128, NG, 8], f32, isOutput=False)
    w_red_d = nc.declare_dram_parameter("w_red", [128, NG, 8, 64], bf16, isOutput=False)
    mp_bias_d = nc.declare_dram_parameter("mp_bias", [64, NG], f32, isOutput=False)
    w_gate_d = nc.declare_dram_parameter("w_gate", [128, NG, NCH, CPW], bf16, isOutput=False)
    bg_bias_d = nc.declare_dram_parameter("bg_bias", [CPW, NCH], f32, isOutput=False)
    w_irfft_d = nc.declare_dram_parameter("w_irfft", [CPW, 2, NCH, 4, 128], bf16, isOutput=False)
    out_d = nc.declare_dram_parameter("out", [BL, H, F], f32, isOutput=True)

    with tile.TileContext(nc) as tc, ExitStack() as ctx:
        consts = ctx.enter_context(tc.tile_pool(name="consts", bufs=1))
        bigio = ctx.enter_context(tc.tile_pool(name="bigio", bufs=1))
        scratch = ctx.enter_context(tc.tile_pool(name="scratch", bufs=1))
        xr_pool = ctx.enter_context(tc.tile_pool(name="xr", bufs=10))
        mp_pool = ctx.enter_context(tc.tile_pool(name="mp", bufs=2))
        gf_pool = ctx.enter_context(tc.tile_pool(name="gf", bufs=2))
        bl_pool = ctx.enter_context(tc.tile_pool(name="bl", bufs=1))

        ps_a = ctx.enter_context(tc.tile_pool(name="ps_a", bufs=3, space="PSUM"))
        ps_proj = ctx.enter_context(tc.tile_pool(name="ps_proj", bufs=3, space="PSUM"))
        ps_red = ctx.enter_context(tc.tile_pool(name="ps_red", bufs=2, space="PSUM"))

        # ---- persistent SBUF tensors ----
        x_sb = bigio.tile([128, 4, BL, F], f32, tag="xout")
        w_rfft_sb = consts.tile([128, 4, 2, NCH, CPW], bf16, tag="w_rfft")
        w_proj_sb = consts.tile([128, NG, 8, 128], bf16, tag="w_proj")
        d_bias_sb = consts.tile([128, NG, 8], f32, tag="d_bias")
        w_red_sb = consts.tile([128, NG, 8, 64], bf16, tag="w_red")
        mp_bias_sb = consts.tile([64, NG], f32, tag="mp_bias")
        w_gate_sb = consts.tile([128, NG, NCH, CPW], bf16, tag="w_gate")
        bg_bias_sb = consts.tile([CPW, NCH], f32, tag="bg_bias")
        w_irfft_sb = consts.tile([CPW, 2, NCH, 4, 128], bf16, tag="w_irfft")

        re_sb = consts.tile([CPW, NCH, COLS], f32, tag="re")
        im_sb = consts.tile([CPW, NCH, COLS], f32, tag="im")
        x_bf = consts.tile([128, 4, BL, F], bf16, tag="x_bf")
        bre_sb = consts.tile([CPW, NCH, COLS], bf16, tag="bre")
        bim_sb = consts.tile([CPW, NCH, COLS], bf16, tag="bim")
        ff_sb = consts.tile([128, NG, COLS], bf16, tag="ff")
        m_sb = consts.tile([CPW, NCH, COLS], f32, tag="m_t")
        ph_sb = consts.tile([CPW, NCH, COLS], f32, tag="ph_t")
        w_sb = consts.tile([CPW, NCH, COLS], f32, tag="w_t")

        # ---- load weights + input ----
        x_r = x_d[:].rearrange("b (kt p) f -> kt p b f", p=128)
        nc.sync.dma_start(out=w_rfft_sb, in_=w_rfft_d[:])
        for kt in range(4):
            nc.sync.dma_start(out=x_sb[:, kt], in_=x_r[kt])
            nc.vector.tensor_copy(out=x_bf[:, kt], in_=x_sb[:, kt])
        nc.sync.dma_start(out=w_proj_sb, in_=w_proj_d[:])
        nc.sync.dma_start(out=d_bias_sb, in_=d_bias_d[:])
        nc.sync.dma_start(out=w_red_sb, in_=w_red_d[:])
        nc.sync.dma_start(out=mp_bias_sb, in_=mp_bias_d[:])
        nc.sync.dma_start(out=w_gate_sb, in_=w_gate_d[:])
        nc.sync.dma_start(out=bg_bias_sb, in_=bg_bias_d[:])
        nc.sync.dma_start(out=w_irfft_sb, in_=w_irfft_d[:])

        # zero FF once (4th rows are never written; junk*0 = NaN risk otherwise)
        nc.vector.memset(ff_sb[:], 0.0)

        # constant bias columns for activations
        def const_col(value, tag):
            t = consts.tile([128, 1], f32, tag=tag)
            nc.vector.memset(t, value)
            return t

        eps_c = const_col(EPS, "c_eps")
        hpi_c = const_col(float(np.pi / 2.0), "c_hpi")
        pi_c = const_col(float(np.pi), "c_pi")

        # ================= rfft =================
        for ch in range(NCH):
            for ri in range(2):
                dst = re_sb if ri == 0 else im_sb
                for nck in range(NC2):
                    pt_f = ps_a.tile([128, 512], f32, tag="ps_a", name="ps_a")
                    pt = pt_f[:CPW]
                    for kt in range(4):
                        nc.tensor.matmul(
                            out=pt,
                            lhsT=w_rfft_sb[:, kt, ri, ch, :],
                            rhs=x_bf[:, kt, 4 * nck:4 * (nck + 1), :],
                            start=(kt == 0),
                            stop=(kt == 3),
                        )
                    nc.vector.tensor_copy(
                        out=dst[:, ch, 512 * nck:512 * (nck + 1)], in_=pt
                    )

        # ================= features: mag, s, c -> FF (interleaved) =============
        for ch in range(NCH):
            re_c = re_sb[:, ch, :]
            im_c = im_sb[:, ch, :]
            msq = scratch.tile([CPW, COLS], f32, tag="msq")
            t2 = scratch.tile([CPW, COLS], f32, tag="tmpb")
            nc.vector.tensor_mul(out=msq, in0=re_c, in1=re_c)
            nc.gpsimd.tensor_mul(out=t2, in0=im_c, in1=im_c)
            nc.vector.tensor_add(out=msq, in0=msq, in1=t2)
            magf = scratch.tile([CPW, COLS], f32, tag="magf")
            nc.scalar.activation(out=magf, in_=msq, func=AF.Sqrt,
                                 bias=eps_c[:CPW], scale=1.0)
            rr = scratch.tile([CPW, COLS], f32, tag="tmpb", name="rr")
            nc.vector.reciprocal_approx_fast(out=rr, in_=magf)
            magb = scratch.tile([CPW, COLS], bf16, tag="magb")
            sbf = scratch.tile([CPW, COLS], bf16, tag="sbf")
            cbf = scratch.tile([CPW, COLS], bf16, tag="cbf")
            nc.scalar.copy(out=magb, in_=magf)
            nc.vector.tensor_mul(out=sbf, in0=im_c, in1=rr)
            nc.vector.tensor_mul(out=cbf, in0=re_c, in1=rr)
            # interleave into FF via SBUF->SBUF DMA (cross-partition)
            for q in range(3):  # groups 3*ch + q, rows 32q..32q+32
                g = 3 * ch + q
                for f, srct in enumerate((magb, sbf, cbf)):
                    nc.sync.dma_start(
                        out=ff_sb[32 * f:32 * f + 32, g, :],
                        in_=srct[32 * q:32 * q + 32, :])

        # ================= per-freq MLP: proj -> relu -> reduce =================
        for g in range(NG):
            ch, p0 = g // 3, 32 * (g % 3)
            for nck in range(NC2):
                red_pt = ps_red.tile([64, 512], f32, tag="ps_red")
                xrs = []
                for j in range(8):
                    proj_pt = ps_proj.tile([128, 512], f32, tag="ps_proj")
                    nc.tensor.matmul(
                        out=proj_pt,
                        lhsT=w_proj_sb[:, g, j, :],
                        rhs=ff_sb[:, g, 512 * nck:512 * (nck + 1)],
                        start=True, stop=True,
                    )
                    xr = xr_pool.tile([128, 512], bf16, tag="xr")
                    if j % 2 == 0:
                        nc.scalar.activation(
                            out=xr, in_=proj_pt, func=AF.Relu,
                            bias=d_bias_sb[:, g, j:j + 1], scale=1.0,
                        )
                    else:
                        nc.vector.tensor_scalar(
                            out=xr, in0=proj_pt,
                            scalar1=d_bias_sb[:, g, j:j + 1], scalar2=0.0,
                            op0=ALU.add, op1=ALU.max,
                        )
                    xrs.append(xr)
                for j in range(8):
                    nc.tensor.matmul(
                        out=red_pt,
                        lhsT=w_red_sb[:, g, j, :],
                        rhs=xrs[j],
                        start=(j == 0), stop=(j == 7),
                    )
                # m / ph activations + relocation
                mp = mp_pool.tile([64, 512], f32, tag="mp")
                nc.scalar.activation(
                    out=mp[0:32, :], in_=red_pt[0:32, :], func=AF.Relu,
                    bias=mp_bias_sb[0:32, g:g + 1], scale=1.0,
                )
                nc.scalar.activation(
                    out=mp[32:64, :], in_=red_pt[32:64, :], func=AF.Sigmoid,
                    bias=mp_bias_sb[32:64, g:g + 1], scale=1.0,
                )
                nc.sync.dma_start(
                    out=m_sb[p0:p0 + 32, ch, 512 * nck:512 * (nck + 1)],
                    in_=mp[0:32, :],
                )
                nc.sync.dma_start(
                    out=ph_sb[p0:p0 + 32, ch, 512 * nck:512 * (nck + 1)],
                    in_=mp[32:64, :],
                )

        # ================= gate =================
        for jt in range(NCH):
            for nck in range(NC2):
                gp_f = ps_a.tile([128, 512], f32, tag="ps_a", name="ps_a")
                gp = gp_f[:CPW]
                for g in range(NG):
                    nc.tensor.matmul(
                        out=gp,
                        lhsT=w_gate_sb[:, g, jt, :],
                        rhs=ff_sb[:, g, 512 * nck:512 * (nck + 1)],
                        start=(g == 0), stop=(g == NG - 1),
                    )
                gt = gf_pool.tile([CPW, 512], f32, tag="gt")
                sg = gf_pool.tile([CPW, 512], f32, tag="sg")
                nc.scalar.activation(out=gt, in_=gp, func=AF.Identity,
                                     bias=bg_bias_sb[:, jt:jt + 1], scale=1.0)
                nc.scalar.activation(out=sg, in_=gp, func=AF.Sigmoid,
                                     bias=bg_bias_sb[:, jt:jt + 1], scale=1.0)
                nc.gpsimd.tensor_mul(out=gt, in0=gt, in1=sg)
                nc.scalar.activation(
                    out=w_sb[:, jt, 512 * nck:512 * (nck + 1)],
                    in_=gt, func=AF.Sigmoid, bias=0.0, scale=1.0,
                )

        # ================= blend =================
        TWO_PI = float(2.0 * np.pi)
        PI = float(np.pi)
        HALF_PI = float(np.pi / 2.0)
        for ch in range(NCH):
            for nck in range(NC2):
                cs = slice(512 * nck, 512 * (nck + 1))
                ph_c = ph_sb[:, ch, cs]
                m_c = m_sb[:, ch, cs]
                w_c = w_sb[:, ch, cs]
                cosph = bl_pool.tile([CPW, 512], f32, tag="cosph")
                sinph = bl_pool.tile([CPW, 512], f32, tag="sinph")
                # Sin spline is only valid on [-pi, pi].
                # sin(2pi u) = sin(pi - 2pi u), arg in [-pi, pi] for u in [0,1].
                # cos(2pi u) = sin(2pi w), w = (u + 0.25) mod 1, same formula.
                # cos(2pi u) = 1 - 2 sin^2(pi u), arg pi*u in [0, pi].
                shalf = bl_pool.tile([CPW, 512], f32, tag="shalf")
                nc.scalar.activation(out=shalf, in_=ph_c, func=AF.Sin,
                                     bias=0.0, scale=PI)
                nc.vector.tensor_mul(out=shalf, in0=shalf, in1=shalf)
                nc.vector.tensor_scalar(out=cosph, in0=shalf, scalar1=-2.0,
                                        scalar2=1.0, op0=ALU.mult, op1=ALU.add)
                nc.scalar.activation(out=sinph, in_=ph_c, func=AF.Sin,
                                     bias=pi_c[:CPW], scale=-TWO_PI)
                for trig, src_t, dst_t, eng in (
                        (cosph, re_sb, bre_sb, nc.vector),
                        (sinph, im_sb, bim_sb, nc.gpsimd)):
                    d_c = src_t[:, ch, cs]
                    t1 = bl_pool.tile([CPW, 512], f32, tag="t1")
                    eng.tensor_mul(out=t1, in0=m_c, in1=trig)   # m*cos
                    eng.tensor_sub(out=t1, in0=t1, in1=d_c)     # - re
                    eng.tensor_mul(out=t1, in0=t1, in1=w_c)     # * w
                    eng.tensor_add(out=dst_t[:, ch, cs], in0=d_c, in1=t1)

        # ================= irfft =================
        out_sb = bigio.tile([128, 4, BL, F], f32, tag="xout")
        for mt in range(4):
            for nck in range(NC2):
                pt = ps_a.tile([128, 512], f32, tag="ps_a")
                k = 0
                for ri, src in enumerate((bre_sb, bim_sb)):
                    for ch in range(NCH):
                        nc.tensor.matmul(
                            out=pt,
                            lhsT=w_irfft_sb[:, ri, ch, mt, :],
                            rhs=src[:, ch, 512 * nck:512 * (nck + 1)],
                            start=(k == 0), stop=(k == 5),
                        )
                        k += 1
                nc.scalar.copy(out=out_sb[:, mt, 4 * nck:4 * (nck + 1), :], in_=pt)

        out_r = out_d[:].rearrange("b (mt p) f -> mt p b f", p=128)
        for mt in range(4):
            nc.sync.dma_start(out=out_r[mt], in_=out_sb[:, mt])

    nc.finalize()
    return nc


_CACHE = {}


def _get_nc():
    if "nc" not in _CACHE:
        _CACHE["nc"] = build_kernel()
    return _CACHE["nc"]


def kernel(x, Wp, bp, Wg, bg, Wm, bm, Wph, bph, _trace=False):
    x = np.ascontiguousarray(np.asarray(x, dtype=np.float32))
    hw = build_host_weights(
        np.asarray(Wp, np.float32), np.asarray(bp, np.float32),
        np.asarray(Wg, np.float32), np.asarray(bg, np.float32),
        np.asarray(Wm, np.float32), np.asarray(bm, np.float32),
        np.asarray(Wph, np.float32), np.asarray(bph, np.float32),
    )
    nc = _get_nc()
    in_maps = []
    for i in range(NCORE):
        m = {"x": np.ascontiguousarray(x[i * BL:(i + 1) * BL])}
        m.update(hw)
        in_maps.append(m)
    res = run_bass_kernel_spmd(nc, in_maps, core_ids=list(range(NCORE)),
                               trace=_trace)
    out = np.concatenate([np.asarray(r["out"]) for r in res.results], axis=0)
    if _trace:
        _CACHE["last_exec_time_ns"] = res.exec_time_ns
        _CACHE["last_results"] = res
    return out.astype(np.float32)
